# revision 3
# baseline (speedup 1.0000x reference)
"""Trainium2 Bass kernel for nn_BatchRelationalWithoutLocationsModule.

Math (per batch b, from the reference):
  o = x[b].reshape(c, h*w).T          # [L, c], c=64, L=256
  A = o @ W0[:c]; B = o @ W0[c:]      # [L, 32]
  h1_{ij} = relu(A_j + B_i + b0); h2 = relu(h1 @ W1 + b1); h3 = relu(h2 @ W2 + b2)
  s = sum_{ij} h3; out = relu(s @ Wp + bp) @ Wo + bo

Distribution: pure data parallel, batch 32 -> 4 per core on 8 NeuronCores.

Device layout per core: partitions = 4 groups x 32 hidden. Group g handles
pair-row i = 64g + t. AT4pb [128, 256] = (A^T + b0) replicated over the 4
groups (bf16); BT4 [128, 64] column t = B^T[:, 64g+t] stacked by group (f32).
Chunk = 4 t-values (1024 pair-columns):
  act1 (x4): h1[:, kL:(k+1)L] = relu(AT4pb + BT4[:, t]) on DVE -- bf16
             in/out hits the 4x DVE perf mode (vs 2x for f32)
  mm1 (x2, N=512, bf16): p1 = W1bd^T h1   (W1bd = blockdiag(W1 x4))
  act2: h2 = relu(p1 + b14) [128, 1024] on ACT (PSUM->SBUF, bf16 out)
  mm2 (x2): p2 = W2bd^T h2
  act3: relu(p2 + b24) + pair-sum: 2/3 of chunks on DVE via
        scalar_tensor_tensor(+bias, max with zeros tile, accum_out),
        1/3 on ACT via activation(Relu, accum_out) -- measured balance.
Both p1 and p2 PSUM pools are double-buffered (4 banks each) so neither
act engine ever waits on a PSUM WAR hazard. All 13 weight tensors are
packed host-side into two DRAM blobs (one bf16, one f32) -> 2 DMAs, so
the x DMAs are not queued behind 13 small weight DMAs at startup.
Then per batch reduce the accum columns, fold the 4 groups with a 0/1
matmul, and run the tiny head MLP on-chip. End-to-end rel err ~1.5e-3
(bf16 h1/h2/weights), tolerance 2e-2.
"""
import sys
sys.path.insert(0, "/opt/trn_rl_repo")
import numpy as np
import ml_dtypes

B, C, HW_, L, H, NG = 32, 64, 16, 256, 32, 4
N_CORES = 8
BPC = B // N_CORES          # batches per core
TPB = L // NG               # 64 t-values per batch
TPC = 4                     # t-values per chunk
NCH = TPB // TPC            # 16 chunks per batch

ACT2_DVE = (0, 1)           # act2 ops on DVE iff gi % den < num
ACT3_DVE = (2, 3)           # act3 ops on DVE iff gi % den < num

_CACHE = {}


def _hit(i, nd):
    n, d = nd
    return (i % d) < n


def _build():
    import concourse.bacc as bacc
    import concourse.mybir as mybir
    import concourse.tile as tile

    f32 = mybir.dt.float32
    bf16 = mybir.dt.bfloat16
    AF = mybir.ActivationFunctionType
    ALU = mybir.AluOpType

    nc = bacc.Bacc("TRN2", target_bir_lowering=False, debug=False)
    P = lambda name, shape, dt=f32, out=False: nc.declare_dram_parameter(
        name, shape, dt, isOutput=out)

    x_in = P("x", [BPC, C, L], bf16)
    wbf_in = P("WBF", [128, 416], bf16)
    wf32_in = P("WF32", [128, 101], f32)
    out = P("out", [H, BPC], out=True)

    with tile.TileContext(nc) as tc:
        with (
            tc.tile_pool(name="wpool", bufs=1) as wpool,
            tc.tile_pool(name="xpool", bufs=2) as xpool,
            tc.tile_pool(name="atpool", bufs=BPC) as atpool,
            tc.tile_pool(name="btpool", bufs=BPC) as btpool,
            tc.tile_pool(name="h1pool", bufs=3) as h1pool,
            tc.tile_pool(name="h2pool", bufs=3) as h2pool,
            tc.tile_pool(name="h3apool", bufs=2) as h3apool,
            tc.tile_pool(name="h3dpool", bufs=2) as h3dpool,
            tc.tile_pool(name="accpool", bufs=2 * BPC) as accpool,
            tc.tile_pool(name="spool", bufs=1) as spool,
            tc.tile_pool(name="ps1", bufs=2, space="PSUM") as ps1_pool,
            tc.tile_pool(name="ps2", bufs=2, space="PSUM") as ps2_pool,
        ):
            xbs = [xpool.tile([C, L], bf16, tag=f"xb{b}", name=f"xb{b}")
                   for b in range(BPC)]
            WBF = wpool.tile([128, 416], bf16, tag="WBF", name="WBF")
            nc.sync.dma_start(WBF[:], wbf_in[:])
            WF = wpool.tile([128, 101], f32, tag="WF", name="WF")
            nc.sync.dma_start(WF[:], wf32_in[:])

            ld = {
                "W0a4": WBF[0:C, 0:128], "W0b": WBF[0:C, 128:160],
                "W1bd": WBF[:, 160:288], "W2bd": WBF[:, 288:416],
                "b04": WF[:, 0:1], "b14": WF[:, 1:2], "b24": WF[:, 2:3],
                "FOLD": WF[:, 3:35], "Wp": WF[0:H, 35:67],
                "bp": WF[0:H, 67:68], "Wo": WF[0:H, 68:100],
                "bo": WF[0:H, 100:101],
            }

            S4 = spool.tile([128, BPC], f32, tag="S4")
            ZT = spool.tile([128, TPC * L], bf16, tag="ZT")
            nc.vector.memset(ZT[:], 0.0)
            ACCs = []
            for b in range(BPC):
                acc_a = accpool.tile([128, NCH], f32, tag="acc",
                                     name=f"accA{b}")
                acc_d = accpool.tile([128, NCH], f32, tag="acc",
                                     name=f"accD{b}")
                nc.vector.memset(acc_a[:], 0.0)
                nc.vector.memset(acc_d[:], 0.0)
                ACCs.append((acc_a, acc_d))

            for b in range(BPC):
                nc.sync.dma_start(xbs[b][:], x_in[b])
            ATs, BTs = [], []
            for b in range(BPC):
                xb = xbs[b]

                pA = ps1_pool.tile([128, L], f32, tag="p1", name="pA")
                nc.tensor.matmul(pA[:], ld["W0a4"][:], xb[:],
                                 start=True, stop=True)
                AT4pb = atpool.tile([128, L], bf16, tag="AT4pb",
                                    name=f"AT4pb{b}")
                # on DVE, not ACT: keeps the prologue off the bottleneck
                # engine (ACT runs act2 + 1/3 of act3 at ~100% busy)
                nc.vector.tensor_scalar(AT4pb[:], pA[:], ld["b04"][:], None,
                                        ALU.add)

                pB = ps2_pool.tile([128, TPB], f32, tag="p2", name="pB")
                for g in range(NG):
                    nc.tensor.matmul(
                        pB[32 * g:32 * (g + 1), :], ld["W0b"][:],
                        xb[:, TPB * g:TPB * (g + 1)],
                        start=True, stop=True, tile_position=(0, 32 * g))
                BT4 = btpool.tile([128, TPB], f32, tag="BT4",
                                  name=f"BT4_{b}")
                nc.vector.tensor_copy(BT4[:], pB[:])

                ATs.append(AT4pb); BTs.append(BT4)

            gi = 0
            W = TPC * L  # 1024
            for ch in range(NCH):
                for b in range(BPC):
                    AT4pb, BT4 = ATs[b], BTs[b]
                    acc_a, acc_d = ACCs[b]

                    h1 = h1pool.tile([128, W], bf16, tag="h1", name="h1")
                    for k in range(TPC):
                        t = ch * TPC + k
                        nc.vector.tensor_scalar(
                            h1[:, k * L:(k + 1) * L], AT4pb[:],
                            BT4[:, t:t + 1], 0.0, ALU.add, ALU.max)

                    p1 = ps1_pool.tile([128, W], f32, tag="p1", name="p1")
                    for m in range(W // 512):
                        nc.tensor.matmul(
                            p1[:, m * 512:(m + 1) * 512], ld["W1bd"][:],
                            h1[:, m * 512:(m + 1) * 512],
                            start=True, stop=True)

                    h2 = h2pool.tile([128, W], bf16, tag="h2", name="h2")
                    if _hit(gi, ACT2_DVE):
                        nc.vector.tensor_scalar(
                            h2[:], p1[:], ld["b14"][:], 0.0, ALU.add, ALU.max)
                    else:
                        nc.scalar.activation(h2[:], p1[:], AF.Relu,
                                             bias=ld["b14"][:])

                    p2 = ps2_pool.tile([128, W], f32, tag="p2", name="p2")
                    for m in range(W // 512):
                        nc.tensor.matmul(
                            p2[:, m * 512:(m + 1) * 512], ld["W2bd"][:],
                            h2[:, m * 512:(m + 1) * 512],
                            start=True, stop=True)

                    if _hit(gi, ACT3_DVE):
                        h3 = h3dpool.tile([128, W], bf16, tag="h3d",
                                          name="h3d")
                        nc.vector.scalar_tensor_tensor(
                            h3[:], p2[:], ld["b24"][:], ZT[:],
                            ALU.add, ALU.max,
                            accum_out=acc_d[:, ch:ch + 1])
                    else:
                        h3 = h3apool.tile([128, W], bf16, tag="h3a",
                                          name="h3a")
                        nc.scalar.activation(
                            h3[:], p2[:], AF.Relu, bias=ld["b24"][:],
                            accum_out=acc_a[:, ch:ch + 1])
                    gi += 1

            for b in range(BPC):
                acc_a, acc_d = ACCs[b]
                sa = spool.tile([128, 1], f32, tag="sa", name="sa")
                sd = spool.tile([128, 1], f32, tag="sd", name="sd")
                nc.vector.tensor_reduce(
                    sa[:], acc_a[:, 0:NCH],
                    axis=mybir.AxisListType.X, op=ALU.add)
                nc.vector.tensor_reduce(
                    sd[:], acc_d[:, 0:NCH],
                    axis=mybir.AxisListType.X, op=ALU.add)
                nc.vector.tensor_tensor(
                    S4[:, b:b + 1], sa[:], sd[:], ALU.add)

            pS = ps1_pool.tile([H, BPC], f32, tag="p1", name="pS")
            nc.tensor.matmul(pS[:], ld["FOLD"][:], S4[:], start=True, stop=True)
            sT = spool.tile([H, BPC], f32, tag="sT", name="sT")
            nc.vector.tensor_copy(sT[:], pS[:])

            pF = ps2_pool.tile([H, BPC], f32, tag="p2", name="pF")
            nc.tensor.matmul(pF[:], ld["Wp"][:], sT[:], start=True, stop=True)
            fT = spool.tile([H, BPC], f32, tag="fT", name="fT")
            nc.scalar.activation(fT[:], pF[:], AF.Relu, bias=ld["bp"][:])

            pO = ps1_pool.tile([H, BPC], f32, tag="p1", name="pO")
            nc.tensor.matmul(pO[:], ld["Wo"][:], fT[:], start=True, stop=True)
            oT = spool.tile([H, BPC], f32, tag="oT", name="oT")
            nc.scalar.activation(oT[:], pO[:], AF.Identity, bias=ld["bo"][:])

            nc.sync.dma_start(out[:], oT[:])

    nc.compile()
    return nc


def _prep_weights(W0, b0, W1, b1, W2, b2, Wp, bp, Wo, bo):
    W0 = np.asarray(W0, np.float32)
    W0a, W0b = W0[:C], W0[C:]
    bd = lambda Wm: np.kron(np.eye(NG, dtype=np.float32),
                            np.asarray(Wm, np.float32))
    wbf = np.zeros((128, 416), np.float32)
    wbf[0:C, 0:128] = np.tile(W0a, (1, NG))
    wbf[0:C, 128:160] = W0b
    wbf[:, 160:288] = bd(W1)
    wbf[:, 288:416] = bd(W2)
    wf32 = np.zeros((128, 101), np.float32)
    wf32[:, 0] = np.tile(np.asarray(b0, np.float32), NG)
    wf32[:, 1] = np.tile(np.asarray(b1, np.float32), NG)
    wf32[:, 2] = np.tile(np.asarray(b2, np.float32), NG)
    wf32[:, 3:35] = np.tile(np.eye(H, dtype=np.float32), (NG, 1))
    wf32[0:H, 35:67] = np.asarray(Wp, np.float32)
    wf32[0:H, 67] = np.asarray(bp, np.float32)
    wf32[0:H, 68:100] = np.asarray(Wo, np.float32)
    wf32[0:H, 100] = np.asarray(bo, np.float32)
    return {"WBF": wbf.astype(ml_dtypes.bfloat16), "WF32": wf32}


def kernel(x_img, W0, b0, W1, b1, W2, b2, Wp, bp, Wo, bo):
    if "nc" not in _CACHE:
        _CACHE["nc"] = _build()
    nc = _CACHE["nc"]

    wd = _prep_weights(W0, b0, W1, b1, W2, b2, Wp, bp, Wo, bo)
    x = np.asarray(x_img, np.float32).reshape(B, C, L).astype(
        ml_dtypes.bfloat16)
    in_maps = [
        {"x": np.ascontiguousarray(x[c * BPC:(c + 1) * BPC]), **wd}
        for c in range(N_CORES)
    ]

    from concourse import bass2jax
    results = bass2jax.run_bass_via_pjrt(nc, in_maps, n_cores=N_CORES)
    full = np.concatenate([r["out"].T for r in results], axis=0)  # [32, 32]
    return full.astype(np.float32)
